# Initial kernel scaffold
#
"""Int8 per-token-quantized linear (MluQuantLinearInt8) on 8 Trainium2 cores.

  out[s, n] = (sum_k q[s,k] * w[n,k]) * x_scale[s] * w_scale[n]
  q = round(x / x_scale) clipped to [-127, 127],  x_scale = max(|x|_row, 1e-8)/127

Sharding: data-parallel over tokens (512/core); weights replicated, streamed
once per core. Weights are host-pretransposed to [K, N] bf16 (int8 values are
exact in bf16, so int8xint8 products accumulate exactly in fp32 PSUM).
Per-core GEMM is weights-stationary: lhsT = wT[128k, 128n] chunks,
rhs = qT[128k, 512tok], out psum [128n, 512tok]; dequant fused into the
PSUM->SBUF eviction; output stored transposed [N, 512] and re-assembled on
host.
"""

import sys
from contextlib import ExitStack
from functools import lru_cache

import numpy as np

for _p in ("/opt/trn_rl_repo", "/root/.axon_site/_ro/trn_rl_repo"):
    if _p not in sys.path:
        sys.path.append(_p)

import ml_dtypes  # noqa: E402

import concourse.bass as bass  # noqa: E402
import concourse.bass2jax as bass2jax  # noqa: E402
import concourse.mybir as mybir  # noqa: E402
import concourse.tile as tile  # noqa: E402
from concourse.bass_utils import (  # noqa: E402
    compile_bir_kernel as _orig_compile_bir_kernel,
    run_bass_kernel_spmd,
)
from concourse.masks import make_identity  # noqa: E402

# The walrus build in this container accepts only ONE sync-wait per
# instruction ("Too many sync wait commands", CoreV3GenImpl setupSyncWait) —
# Tile's kernel-tail drain carries several. Split extra waits onto preceding
# single-wait EventSemaphore carriers on the same engine (engine program order
# makes the AND of waits equivalent).
import json as _json  # noqa: E402


def _split_multi_waits(bir_json):
    d = _json.loads(bir_json)
    changed = False
    for fn in d.get("functions", []):
        for bb in fn.get("blocks", []) or []:
            insts = bb.get("instructions")
            if not insts:
                continue
            out = []
            for ins in insts:
                si = ins.get("sync_info")
                waits = (si or {}).get("on_wait") or []
                if len(waits) > 1:
                    for j, w in enumerate(waits[:-1]):
                        out.append(
                            {
                                "engine": ins.get("engine"),
                                "ins": [],
                                "outs": [],
                                "name": f"{ins.get('name', 'I')}_w{j}",
                                "opcode": "EventSemaphore",
                                "sync_info": {"on_update": [], "on_wait": [w]},
                            }
                        )
                    si["on_wait"] = [waits[-1]]
                    changed = True
                out.append(ins)
            bb["instructions"] = out
    if not changed:
        return bir_json
    return _json.dumps(d).encode()


def _patched_compile_bir_kernel(bir_json, tmpdir, neff_name="file.neff"):
    return _orig_compile_bir_kernel(
        _split_multi_waits(bir_json), tmpdir, neff_name=neff_name
    )


bass2jax.compile_bir_kernel = _patched_compile_bir_kernel

P = 128
NCORES = 8
S, K_FULL, N_FULL = 4096, 4096, 16384
QMAX = 127.0
MAGIC = 12582912.0  # 1.5 * 2**23: (y + MAGIC) - MAGIC == RNE-round(y) for |y| < 2**22
F32 = mybir.dt.float32
BF16 = mybir.dt.bfloat16


def build_nc(S_C, K, N, NSUB=4, exact_divide=True):
    """One-core program; SPMD-replicated across cores by the runner.

    Inputs (per core):
      x   [S_C, K]  f32 - this core's token slice
      wt  [WC, P, KC, NSUB*P] bf16 - weights, host-packed as SBUF-layout chunks
      ws  [P, NT]   f32 - weight_scale packed ws[p, nt] = weight_scale[nt*128+p]
    Output:
      outT [N, S_C] f32 - dequantized output, transposed
    """
    KC = K // P  # contraction chunks
    TT = S_C // P  # token tiles
    NT = N // P  # output-channel tiles (one psum tile each)
    WC = NT // NSUB  # streamed weight chunks

    nc = bass.Bass()
    x = nc.declare_dram_parameter("x", [S_C, K], F32, isOutput=False)
    wt = nc.declare_dram_parameter("wt", [WC, P, KC, NSUB * P], BF16, isOutput=False)
    ws = nc.declare_dram_parameter("ws", [P, NT], F32, isOutput=False)
    outT = nc.declare_dram_parameter("outT", [N, S_C], F32, isOutput=True)
    xs_scratch = nc.dram_tensor("xs_scratch", [S_C], F32)

    outT_t = outT.rearrange("(nt p) s -> nt p s", p=P)

    with tile.TileContext(nc) as tc, ExitStack() as ctx:
        const_pool = ctx.enter_context(tc.tile_pool(name="const", bufs=1))
        xpool = ctx.enter_context(tc.tile_pool(name="xp", bufs=4))
        qpool = ctx.enter_context(tc.tile_pool(name="qp", bufs=3))
        qt_pool = ctx.enter_context(tc.tile_pool(name="qt", bufs=1))
        wpool = ctx.enter_context(tc.tile_pool(name="wp", bufs=2))
        opool = ctx.enter_context(tc.tile_pool(name="op", bufs=4))
        spool = ctx.enter_context(tc.tile_pool(name="sp", bufs=1))
        pt_pool = ctx.enter_context(tc.tile_pool(name="ptp", bufs=1, space="PSUM"))
        ps_pool = ctx.enter_context(tc.tile_pool(name="psp", bufs=7, space="PSUM"))

        ident_f32 = const_pool.tile([P, P], F32)
        make_identity(nc, ident_f32)

        ws_sb = const_pool.tile([P, NT], F32)
        nc.sync.dma_start(ws_sb, ws[:, :])

        # ---- Phase 1: per-token dynamic int8 quantization + transpose ----
        # qT[k%128, t, k//128, tok%128]: each token tile's DMA-transpose target
        # is per-partition contiguous (non-contiguous dst breaks DMA transpose)
        qT = qt_pool.tile([P, TT, KC, P], BF16)
        xs_all = spool.tile([P, TT], F32)  # xs_all[p, t] = x_scale[t*128+p]

        def load_wchunk(wc):
            wtile = wpool.tile([P, KC, NSUB * P], BF16, tag="wtile")
            half = KC // 2
            nc.sync.dma_start(wtile[:, :half], wt[wc, :, :half])
            nc.sync.dma_start(wtile[:, half:], wt[wc, :, half:])
            return wtile

        # weight chunk 0 FIRST: it must be resident the moment qT completes,
        # and the x loads + XBAR transposes otherwise saturate DMA ahead of it
        # (measured 37.7us PE stall when it queued behind them)
        wtiles = {0: load_wchunk(0)}

        # then all token-tile loads (they pace the quant chain)
        xts = []
        for t in range(TT):
            xt = xpool.tile([P, K], F32)
            nc.sync.dma_start(xt, x[t * P : (t + 1) * P, :])
            xts.append(xt)

        for t in range(TT):
            xt = xts[t]
            amax = spool.tile([P, 1], F32, tag="amax")
            nc.vector.tensor_reduce(
                out=amax,
                in_=xt,
                axis=mybir.AxisListType.X,
                op=mybir.AluOpType.max,
                apply_absolute_value=True,
            )
            # amax' = max(amax, 1e-8); x_scale = amax'/127 (~1ulp, via *1/127);
            # q = round(x * (127 * recip(amax'))) - DVE has no divide, but
            # reciprocal is bit-exact; the ~1ulp quantizer error flips a
            # rounding boundary on ~0.1 elements per 4096-row (negligible).
            nc.vector.tensor_scalar(
                amax, amax, 1e-8, None, op0=mybir.AluOpType.max
            )
            nc.vector.tensor_scalar(
                xs_all[:, t : t + 1],
                amax,
                float(np.float32(1.0 / 127.0)),
                None,
                op0=mybir.AluOpType.mult,
            )
            inv = spool.tile([P, 1], F32, tag="inv")
            nc.vector.reciprocal(inv, amax)
            nc.vector.tensor_scalar(
                inv, inv, QMAX, None, op0=mybir.AluOpType.mult
            )
            # y = x*inv + MAGIC on the (otherwise idle) scalar engine
            nc.scalar.activation(
                xt, xt, mybir.ActivationFunctionType.Copy, bias=MAGIC, scale=inv
            )
            q = qpool.tile([P, K], BF16)
            nc.vector.tensor_scalar(
                q, xt, MAGIC, None, op0=mybir.AluOpType.subtract
            )
            # whole-tile transpose on the DMA xbar: [tok, (kc ki)] -> [ki, kc, tok]
            nc.sync.dma_start(qT[:, t], q, transpose=True)

        # chunk 1 prefetch deferred to here so it doesn't delay the x loads
        if WC > 1:
            wtiles[1] = load_wchunk(1)

        # ---- xs broadcast tile [p, tok] = x_scale[tok] (via transpose+DMA) ----
        xs_rowT_ps = pt_pool.tile([TT, P], F32, tag="xs_t")
        nc.tensor.transpose(xs_rowT_ps, xs_all, ident_f32)
        xs_rowT = spool.tile([TT, P], F32, tag="xs_rowT")
        nc.vector.tensor_copy(xs_rowT, xs_rowT_ps)
        nc.sync.dma_start(xs_scratch.rearrange("(t p) -> t p", p=P), xs_rowT)
        xsb = spool.tile([P, S_C], F32, tag="xsb")
        nc.sync.dma_start(xsb, xs_scratch[None, :].to_broadcast((P, S_C)))

        # ---- Phase 2: streamed weights-stationary GEMM + fused dequant ----
        for wc in range(WC):
            wtile = wtiles.pop(wc) if wc in wtiles else load_wchunk(wc)
            for sub in range(NSUB):
                nt = wc * NSUB + sub
                ps = ps_pool.tile([P, S_C], F32)
                for kc in range(KC):
                    nc.tensor.matmul(
                        ps,
                        lhsT=wtile[:, kc, sub * P : (sub + 1) * P],
                        rhs=qT[:, :, kc, :],
                        start=(kc == 0),
                        stop=(kc == KC - 1),
                    )
                out_sb = opool.tile([P, S_C], F32)
                # out = (acc * w_scale[n]) * x_scale[tok]
                nc.vector.scalar_tensor_tensor(
                    out=out_sb,
                    in0=ps,
                    scalar=ws_sb[:, nt : nt + 1],
                    in1=xsb,
                    op0=mybir.AluOpType.mult,
                    op1=mybir.AluOpType.mult,
                )
                nc.sync.dma_start(outT_t[nt], out_sb)

    return nc


def pack_inputs(input_tensor, weight, weight_scale, S_C, K, N, NSUB=4):
    """Host-side prep: shard x, pack weights to bf16 SBUF-chunk layout."""
    KC = K // P
    NT = N // P
    WC = NT // NSUB
    x = np.ascontiguousarray(input_tensor.reshape(-1, K))  # [S, K]
    w_bf = weight.astype(ml_dtypes.bfloat16)  # [N, K], int8 values exact
    # pack[wc, p, kc, n] = w[wc*NSUB*P + n, kc*P + p]
    wt = np.ascontiguousarray(
        w_bf.reshape(WC, NSUB * P, KC, P).transpose(0, 3, 2, 1)
    )
    ws = np.ascontiguousarray(
        weight_scale.reshape(NT, P).T.astype(np.float32)
    )  # [P, NT]
    return x, wt, ws


@lru_cache(maxsize=2)
def _compiled_nc(S_C, K, N, NSUB, exact_divide):
    return build_nc(S_C, K, N, NSUB=NSUB, exact_divide=exact_divide)


def run(input_tensor, weight, weight_scale, n_cores=NCORES, trace=False,
        exact_divide=True, NSUB=4):
    Sfull, K = input_tensor.shape[-2], input_tensor.shape[-1]
    N = weight.shape[0]
    S_C = Sfull // n_cores
    x, wt, ws = pack_inputs(input_tensor, weight, weight_scale, S_C, K, N, NSUB)
    nc = _compiled_nc(S_C, K, N, NSUB, exact_divide)
    in_maps = [
        {"x": np.ascontiguousarray(x[c * S_C : (c + 1) * S_C]), "wt": wt, "ws": ws}
        for c in range(n_cores)
    ]
    res = run_bass_kernel_spmd(nc, in_maps, core_ids=list(range(n_cores)), trace=trace)
    out = np.empty((Sfull, N), np.float32)
    for c in range(n_cores):
        out[c * S_C : (c + 1) * S_C] = res.results[c]["outT"].T
    return out[None], res


def kernel(input_tensor, weight, weight_scale):
    out, _ = run(
        np.asarray(input_tensor), np.asarray(weight), np.asarray(weight_scale)
    )
    return out



# revision 40
# speedup vs baseline: 1.0222x; 1.0222x over previous
"""Int8 per-token-quantized linear (MluQuantLinearInt8) on 8 Trainium2 cores.

  out[s, n] = (sum_k q[s,k] * w[n,k]) * x_scale[s] * w_scale[n]
  q = round(x / x_scale) clipped to [-127, 127],  x_scale = max(|x|_row, 1e-8)/127

Sharding: data-parallel over tokens (512/core); weights replicated, streamed
once per core. Weights are host-pretransposed f16 (int8 values exact, so
int8xint8 products accumulate exactly in fp32 PSUM). Quantization is one
engine pass per half-tile: q' = f16(x*inv + 1536) -- the f16 convert IS the
RNE-to-integer rounding (1536+q sits in fp16's ulp-1 band) -- and the +1536
bias is removed at PSUM eviction via the host-precomputed 1536*colsum(w).
Per-core GEMM is weights-stationary: lhsT = wT[128k, 128n] chunks,
rhs = qT[128k, 512tok], out psum [128n, 512tok]; dequant fused into the
PSUM->SBUF eviction; output stored transposed [N, 512] and re-assembled on
host.
"""

import sys
from contextlib import ExitStack
from functools import lru_cache

import numpy as np

for _p in ("/opt/trn_rl_repo", "/root/.axon_site/_ro/trn_rl_repo"):
    if _p not in sys.path:
        sys.path.append(_p)

import ml_dtypes  # noqa: E402

import concourse.bass as bass  # noqa: E402
import concourse.bass2jax as bass2jax  # noqa: E402
import concourse.mybir as mybir  # noqa: E402
import concourse.tile as tile  # noqa: E402
from concourse.bass_utils import (  # noqa: E402
    compile_bir_kernel as _orig_compile_bir_kernel,
    run_bass_kernel_spmd,
)
from concourse.masks import make_identity  # noqa: E402

# The walrus build in this container accepts only ONE sync-wait per
# instruction ("Too many sync wait commands", CoreV3GenImpl setupSyncWait) —
# Tile's kernel-tail drain carries several. Split extra waits onto preceding
# single-wait EventSemaphore carriers on the same engine (engine program order
# makes the AND of waits equivalent).
import json as _json  # noqa: E402


def _split_multi_waits(bir_json):
    d = _json.loads(bir_json)
    changed = False
    for fn in d.get("functions", []):
        for bb in fn.get("blocks", []) or []:
            insts = bb.get("instructions")
            if not insts:
                continue
            out = []
            for ins in insts:
                si = ins.get("sync_info")
                waits = (si or {}).get("on_wait") or []
                if len(waits) > 1:
                    for j, w in enumerate(waits[:-1]):
                        out.append(
                            {
                                "engine": ins.get("engine"),
                                "ins": [],
                                "outs": [],
                                "name": f"{ins.get('name', 'I')}_w{j}",
                                "opcode": "EventSemaphore",
                                "sync_info": {"on_update": [], "on_wait": [w]},
                            }
                        )
                    si["on_wait"] = [waits[-1]]
                    changed = True
                out.append(ins)
            bb["instructions"] = out
    if not changed:
        return bir_json
    return _json.dumps(d).encode()


def _patched_compile_bir_kernel(bir_json, tmpdir, neff_name="file.neff"):
    return _orig_compile_bir_kernel(
        _split_multi_waits(bir_json), tmpdir, neff_name=neff_name
    )


bass2jax.compile_bir_kernel = _patched_compile_bir_kernel

P = 128
NCORES = 8
S, K_FULL, N_FULL = 4096, 4096, 16384
QMAX = 127.0
# fp16 magic: 1536+q is exact in fp16 for q in [-127,127] (ulp-1 band
# [1024,2048)), so f16(x*inv + 1536) IS the RNE integer quantizer, offset by
# 1536. The offset flows through the GEMM as 1536*colsum(w) per out-channel
# and is subtracted at PSUM eviction.
M16 = 1536.0
F32 = mybir.dt.float32
BF16 = mybir.dt.bfloat16
F16 = mybir.dt.float16


def build_nc(S_C, K, N, NSUB=4, exact_divide=True):
    """One-core program; SPMD-replicated across cores by the runner.

    Inputs (per core):
      x    [S_C, K]  f32 - this core's token slice
      wt   [WC, P, KC, NSUB*P] f16 - weights, host-packed as SBUF-layout chunks
      ws   [P, NT]   f32 - weight_scale packed ws[p, nt] = weight_scale[nt*128+p]
      csum [P, NT]   f32 - 1536*colsum(w) packed the same way
    Output:
      outT [N, S_C] f32 - dequantized output, transposed
    """
    KC = K // P  # contraction chunks
    TT = S_C // P  # token tiles
    NT = N // P  # output-channel tiles (one psum tile each)
    WC = NT // NSUB  # streamed weight chunks

    nc = bass.Bass()
    x = nc.declare_dram_parameter("x", [S_C, K], F32, isOutput=False)
    wt = nc.declare_dram_parameter("wt", [WC, P, KC, NSUB * P], F16, isOutput=False)
    ws = nc.declare_dram_parameter("ws", [P, NT], F32, isOutput=False)
    csum = nc.declare_dram_parameter("csum", [P, NT], F32, isOutput=False)
    outT = nc.declare_dram_parameter("outT", [N, S_C], F32, isOutput=True)
    xs_scratch = nc.dram_tensor("xs_scratch", [S_C], F32)

    outT_t = outT.rearrange("(nt p) s -> nt p s", p=P)

    KH = K // 2  # half-tile K extent (quant pipeline granularity)
    HKC = KC // 2

    with tile.TileContext(nc) as tc, ExitStack() as ctx:
        const_pool = ctx.enter_context(tc.tile_pool(name="const", bufs=1))
        xpool = ctx.enter_context(tc.tile_pool(name="xp", bufs=6))
        qpool = ctx.enter_context(tc.tile_pool(name="qp", bufs=6))
        qt_pool = ctx.enter_context(tc.tile_pool(name="qt", bufs=1))
        wpool = ctx.enter_context(tc.tile_pool(name="wp", bufs=2))
        opool = ctx.enter_context(tc.tile_pool(name="op", bufs=3))
        spool = ctx.enter_context(tc.tile_pool(name="sp", bufs=1))
        pt_pool = ctx.enter_context(tc.tile_pool(name="ptp", bufs=1, space="PSUM"))
        ps_pool = ctx.enter_context(tc.tile_pool(name="psp", bufs=7, space="PSUM"))

        ident_f32 = const_pool.tile([P, P], F32)
        make_identity(nc, ident_f32)

        # Preload the scalar engine's activation table during the preamble so
        # the first real quant doesn't pay the ~1.3us ACT_TABLE_LOAD.
        act_warm = const_pool.tile([P, 1], F32, tag="actwarm")
        nc.scalar.activation(
            act_warm, ident_f32[:, :1], mybir.ActivationFunctionType.Copy,
            bias=0.0, scale=1.0,
        )

        # ---- Phase 1: per-token dynamic int8 quantization + transpose ----
        # qT[k%128, t, k//128, tok%128]: each transpose target slice is
        # per-partition contiguous (non-contiguous dst breaks DMA transpose)
        qT = qt_pool.tile([P, TT, KC, P], F16)
        xs_all = spool.tile([P, TT], F32)  # xs_all[p, t] = x_scale[t*128+p]

        # x half-tiles FIRST in DMA issue order: they pace everything.
        xh = []
        for t in range(TT):
            row = []
            for h in range(2):
                xth = xpool.tile([P, KH], F32)
                nc.sync.dma_start(
                    xth, x[t * P : (t + 1) * P, h * KH : (h + 1) * KH]
                )
                row.append(xth)
            xh.append(row)

        ws_sb = const_pool.tile([P, NT], F32)
        nc.sync.dma_start(ws_sb, ws[:, :])
        csum_sb = const_pool.tile([P, NT], F32)
        nc.sync.dma_start(csum_sb, csum[:, :])

        def load_wchunk(wc):
            wtile = wpool.tile([P, KC, NSUB * P], F16, tag="wtile")
            half = KC // 2
            nc.sync.dma_start(wtile[:, :half], wt[wc, :, :half])
            nc.sync.dma_start(wtile[:, half:], wt[wc, :, half:])
            return wtile

        wtiles = {}

        # amax per tile: per-half vector reduces (start on first-half arrival)
        # combined with a [P,1] max. Tile 0 reduces quarter-grain so inv_t0
        # (which heads the whole quant+transpose chain) lands ~2us earlier.
        invs = []
        for t in range(TT):
            pam0 = spool.tile([P, 1], F32, tag=f"pam0_{t}")
            amax = spool.tile([P, 1], F32, tag=f"amax_{t}")
            if t == 0:
                pq = spool.tile([P, 1], F32, tag="pam0q")
                nc.vector.tensor_reduce(
                    out=pam0, in_=xh[t][0][:, : KH // 2],
                    axis=mybir.AxisListType.X,
                    op=mybir.AluOpType.max, apply_absolute_value=True,
                )
                nc.vector.tensor_reduce(
                    out=pq, in_=xh[t][0][:, KH // 2 :],
                    axis=mybir.AxisListType.X,
                    op=mybir.AluOpType.max, apply_absolute_value=True,
                )
                nc.vector.tensor_tensor(pam0, pam0, pq, op=mybir.AluOpType.max)
                nc.vector.tensor_reduce(
                    out=amax, in_=xh[t][1][:, : KH // 2],
                    axis=mybir.AxisListType.X,
                    op=mybir.AluOpType.max, apply_absolute_value=True,
                )
                nc.vector.tensor_reduce(
                    out=pq, in_=xh[t][1][:, KH // 2 :],
                    axis=mybir.AxisListType.X,
                    op=mybir.AluOpType.max, apply_absolute_value=True,
                )
                nc.vector.tensor_tensor(amax, amax, pq, op=mybir.AluOpType.max)
            else:
                nc.vector.tensor_reduce(
                    out=pam0,
                    in_=xh[t][0],
                    axis=mybir.AxisListType.X,
                    op=mybir.AluOpType.max,
                    apply_absolute_value=True,
                )
                nc.vector.tensor_reduce(
                    out=amax,
                    in_=xh[t][1],
                    axis=mybir.AxisListType.X,
                    op=mybir.AluOpType.max,
                    apply_absolute_value=True,
                )
            # amax' = max(amax, 1e-8); x_scale = amax'/127 (~1ulp, via *1/127);
            # q = round(x * (127 * recip(amax'))) - DVE has no divide, but
            # reciprocal is bit-exact; the ~1ulp quantizer error flips a
            # rounding boundary on ~0.1 elements per 4096-row (negligible).
            nc.vector.tensor_tensor(amax, amax, pam0, op=mybir.AluOpType.max)
            nc.vector.tensor_scalar(
                amax, amax, 1e-8, None, op0=mybir.AluOpType.max
            )
            nc.vector.tensor_scalar(
                xs_all[:, t : t + 1],
                amax,
                float(np.float32(1.0 / 127.0)),
                None,
                op0=mybir.AluOpType.mult,
            )
            inv = spool.tile([P, 1], F32, tag=f"inv_{t}")
            nc.vector.reciprocal(inv, amax)
            nc.vector.tensor_scalar(
                inv, inv, QMAX, None, op0=mybir.AluOpType.mult
            )
            invs.append(inv)

        # One-pass quant: q' = f16(x*inv + 1536) (the f16 convert IS the RNE
        # integer rounding). Scalar engine does most halves; the last tile's
        # second half goes on vector, which is free right after its reduces.
        # Transposes go through a serial sync-sequencer dispatch path
        # (~0.6us/dispatch + ~1.4us/0.25MiB execute), so issue them half-grain
        # (fewest dispatches) and slot the bulky weight loads BETWEEN
        # transpose groups; the phase-2 weight stream (issued later) is
        # head-of-line-gated behind everything quant needs.
        def quant_tile(t):
            for h in range(2):
                q = qpool.tile([P, KH], F16)
                if t == TT - 1 and h == 1:
                    nc.vector.tensor_scalar(
                        q, xh[t][h], invs[t], M16,
                        op0=mybir.AluOpType.mult, op1=mybir.AluOpType.add,
                    )
                else:
                    nc.scalar.activation(
                        q, xh[t][h], mybir.ActivationFunctionType.Copy,
                        bias=M16, scale=invs[t],
                    )
                nc.sync.dma_start(
                    qT[:, t, h * HKC : (h + 1) * HKC, :], q, transpose=True
                )

        quant_tile(0)
        quant_tile(1)
        wtiles[0] = load_wchunk(0)
        quant_tile(2)
        quant_tile(3)
        if WC > 1:
            wtiles[1] = load_wchunk(1)

        # ---- xs broadcast tile [p, tok] = x_scale[tok] (via transpose+DMA) ----
        xs_rowT_ps = pt_pool.tile([TT, P], F32, tag="xs_t")
        nc.tensor.transpose(xs_rowT_ps, xs_all, ident_f32)
        xs_rowT = spool.tile([TT, P], F32, tag="xs_rowT")
        nc.vector.tensor_copy(xs_rowT, xs_rowT_ps)
        nc.sync.dma_start(xs_scratch.rearrange("(t p) -> t p", p=P), xs_rowT)
        xsb = spool.tile([P, S_C], F32, tag="xsb")
        nc.sync.dma_start(xsb, xs_scratch[None, :].to_broadcast((P, S_C)))

        # ---- Phase 2: streamed weights-stationary GEMM + fused dequant ----
        def evict(nt, ps):
            out_sb = opool.tile([P, S_C], F32)
            # acc = psum - 1536*colsum(w);  out = (acc * w_scale) * x_scale
            nc.vector.tensor_scalar(
                out_sb, ps, csum_sb[:, nt : nt + 1], None,
                op0=mybir.AluOpType.subtract,
            )
            nc.vector.scalar_tensor_tensor(
                out=out_sb,
                in0=out_sb,
                scalar=ws_sb[:, nt : nt + 1],
                in1=xsb,
                op0=mybir.AluOpType.mult,
                op1=mybir.AluOpType.mult,
            )
            nc.sync.dma_start(outT_t[nt], out_sb)

        # wc0 runs token-split (A: tokens 0-255, B: 256-511): each half is a
        # complete accumulation group in its own psum tile, evicted into the
        # matching outT column slice. 256-wide matmuls run back-to-back at
        # full rate, so the split costs nothing and lets the PE start at the
        # earliest moment the early transposes allow.
        TS = S_C // 2

        def evict_half(nt, ps, half):
            sl = slice(half * TS, (half + 1) * TS)
            out_sb = opool.tile([P, TS], F32, tag="osplit")
            nc.vector.tensor_scalar(
                out_sb, ps[:, :TS], csum_sb[:, nt : nt + 1], None,
                op0=mybir.AluOpType.subtract,
            )
            nc.vector.scalar_tensor_tensor(
                out=out_sb,
                in0=out_sb,
                scalar=ws_sb[:, nt : nt + 1],
                in1=xsb[:, sl],
                op0=mybir.AluOpType.mult,
                op1=mybir.AluOpType.mult,
            )
            nc.sync.dma_start(outT_t[nt][:, sl], out_sb)

        wtile0 = wtiles.pop(0)
        for half in range(2):
            for sub in range(NSUB):
                ps = ps_pool.tile([P, S_C], F32)
                for kc in range(KC):
                    nc.tensor.matmul(
                        ps[:, :TS],
                        lhsT=wtile0[:, kc, sub * P : (sub + 1) * P],
                        rhs=qT[:, 2 * half : 2 * half + 2, kc, :],
                        start=(kc == 0),
                        stop=(kc == KC - 1),
                    )
                evict_half(sub, ps, half)

        for wc in range(1, WC):
            wtile = wtiles.pop(wc) if wc in wtiles else load_wchunk(wc)
            for sub in range(NSUB):
                nt = wc * NSUB + sub
                if wc == WC - 1 and sub == NSUB - 1:
                    # last output tile token-split: the first half's eviction
                    # and store overlap the second half's matmuls, halving
                    # the post-GEMM drain.
                    for half in range(2):
                        ps = ps_pool.tile([P, S_C], F32)
                        for kc in range(KC):
                            nc.tensor.matmul(
                                ps[:, :TS],
                                lhsT=wtile[:, kc, sub * P : (sub + 1) * P],
                                rhs=qT[:, 2 * half : 2 * half + 2, kc, :],
                                start=(kc == 0),
                                stop=(kc == KC - 1),
                            )
                        evict_half(nt, ps, half)
                    continue
                ps = ps_pool.tile([P, S_C], F32)
                for kc in range(KC):
                    nc.tensor.matmul(
                        ps,
                        lhsT=wtile[:, kc, sub * P : (sub + 1) * P],
                        rhs=qT[:, :, kc, :],
                        start=(kc == 0),
                        stop=(kc == KC - 1),
                    )
                evict(nt, ps)

    return nc


def pack_inputs(input_tensor, weight, weight_scale, S_C, K, N, NSUB=4):
    """Host-side prep: shard x, pack weights to f16 SBUF-chunk layout."""
    KC = K // P
    NT = N // P
    WC = NT // NSUB
    x = np.ascontiguousarray(input_tensor.reshape(-1, K))  # [S, K]
    w_f16 = weight.astype(np.float16)  # [N, K], int8 values exact
    # pack[wc, p, kc, n] = w[wc*NSUB*P + n, kc*P + p]
    wt = np.ascontiguousarray(
        w_f16.reshape(WC, NSUB * P, KC, P).transpose(0, 3, 2, 1)
    )
    ws = np.ascontiguousarray(
        weight_scale.reshape(NT, P).T.astype(np.float32)
    )  # [P, NT]
    # static per-out-channel offset from the +1536 activation bias:
    # 1536*colsum is an exact f32 (multiple of 512, < 2**25)
    colsum = weight.astype(np.int64).sum(axis=1).astype(np.float64)
    csum = np.ascontiguousarray(
        (M16 * colsum).astype(np.float32).reshape(NT, P).T
    )  # [P, NT]
    return x, wt, ws, csum


@lru_cache(maxsize=2)
def _compiled_nc(S_C, K, N, NSUB, exact_divide):
    return build_nc(S_C, K, N, NSUB=NSUB, exact_divide=exact_divide)


def run(input_tensor, weight, weight_scale, n_cores=NCORES, trace=False,
        exact_divide=True, NSUB=4):
    Sfull, K = input_tensor.shape[-2], input_tensor.shape[-1]
    N = weight.shape[0]
    S_C = Sfull // n_cores
    x, wt, ws, csum = pack_inputs(
        input_tensor, weight, weight_scale, S_C, K, N, NSUB
    )
    nc = _compiled_nc(S_C, K, N, NSUB, exact_divide)
    in_maps = [
        {
            "x": np.ascontiguousarray(x[c * S_C : (c + 1) * S_C]),
            "wt": wt,
            "ws": ws,
            "csum": csum,
        }
        for c in range(n_cores)
    ]
    res = run_bass_kernel_spmd(nc, in_maps, core_ids=list(range(n_cores)), trace=trace)
    out = np.empty((Sfull, N), np.float32)
    for c in range(n_cores):
        out[c * S_C : (c + 1) * S_C] = res.results[c]["outT"].T
    return out[None], res


def kernel(input_tensor, weight, weight_scale):
    out, _ = run(
        np.asarray(input_tensor), np.asarray(weight), np.asarray(weight_scale)
    )
    return out

